# revision 2
# baseline (speedup 1.0000x reference)
"""BitSSM fused kernel for 8 Trainium2 NeuronCores.

Strategy
--------
Data-parallel over tokens: B*S = 16384 tokens split into 8 shards of 2048.
All ops are token-local except the causal depthwise conv (K=4), whose
3-token left halo is precomputed on the host per shard.

All three GEMMs run as fp8e4m3 x fp8e4m3 DoubleRow matmuls (2 contraction
planes per pass).  BitNet ternary weights are exact in fp8.  Activation
operands are split hi+lo into two fp8 tensors where full precision is
needed (in_proj input x, out_proj input y); the x_proj operand xc is a
single scaled fp8 since the sigmoid squashes its quantization error.
xc and y are scaled by 64 before fp8 quantization to stay out of the fp8
subnormal range; the 1/64 folds into the activation scale immediates.

Per-core phases (weight-stationary loops amortize PE weight loads):
  A: in_proj   psum[ct,j] = sum_kp Wi_kp^T [x_hi;x_lo]   (DoubleRow fp8)
     conv      xi = fp16(psum); 4 tap muls + 3 adds on DVE (fp16 4x/2x)
     silu      xc = silu(conv + bc)            (ACT, fp16)
     quant     xc8 = fp8(64*xc); xcE8 = fp8(64*xc - xc8)
  B: x_proj    psum[c2,j] = sum_kp Wx_kp^T xc8           (DoubleRow fp8)
     gate      g = sigmoid(s_x/64 * psum + bx)  (ACT, fp16)
     y         y = (xc8 + xcE8) * g; y_hi = fp8(y); y_lo = fp8(y - y_hi)
  C: out_proj  psum[dt,j] = sum_kp Wo_kp^T [y_hi;y_lo]   (DoubleRow fp8)
     out       out = Identity(s_out/64 * psum + bo) -> bf16 -> DMA
"""

import sys

if '/opt/trn_rl_repo' not in sys.path:
    sys.path.insert(0, '/opt/trn_rl_repo')

import numpy as np
import ml_dtypes

D_MODEL, D_STATE, D_INNER = 1024, 2048 // 128, 2048
D_STATE = 16
EPS = 1e-5
B, S = 4, 4096
N_CORES = 8
T = (B * S) // N_CORES          # tokens per core (2048)
W = 512                         # psum tile width (tokens)
NJ = T // W                     # 4 psum tiles per channel-plane
KI = D_MODEL // 128             # 8 contraction planes for in_proj
KC = D_INNER // 128             # 16 contraction planes for x/out_proj
CT = D_INNER // 128             # 16 channel planes of d_inner
DT = D_MODEL // 128             # 8 channel planes of d_model
SC = 64.0                       # fp8 scale for xc / y

_BUILD_CACHE = {}


def _build(s_x: float, s_out: float):
    import concourse.tile as tile
    from concourse import bacc, mybir

    nc = bacc.Bacc("TRN2", target_bir_lowering=False, debug=False)
    f32 = mybir.dt.float32
    fp16 = mybir.dt.float16
    bf16 = mybir.dt.bfloat16
    fp8 = mybir.dt.float8e4
    AF = mybir.ActivationFunctionType
    ALU = mybir.AluOpType
    DR = mybir.MatmulPerfMode.DoubleRow

    xhl_d = nc.dram_tensor("xhl", [128, 2 * KI * T], fp8, kind="ExternalInput")
    wi_d = nc.dram_tensor("wi", [128, KI * D_INNER], fp8, kind="ExternalInput")
    wx_d = nc.dram_tensor("wx", [128, KC * D_INNER], fp8, kind="ExternalInput")
    wo_d = nc.dram_tensor("wo", [128, KC * D_MODEL], fp8, kind="ExternalInput")
    wc_d = nc.dram_tensor("wc", [128, CT * 4], f32, kind="ExternalInput")
    bc_d = nc.dram_tensor("bc", [128, CT], f32, kind="ExternalInput")
    bx_d = nc.dram_tensor("bx", [128, CT], f32, kind="ExternalInput")
    bo_d = nc.dram_tensor("bo", [128, DT], f32, kind="ExternalInput")
    h0_d = nc.dram_tensor("h0", [128, CT * 3], f32, kind="ExternalInput")
    out_d = nc.dram_tensor("out", [128, DT * T], bf16, kind="ExternalOutput")

    with tile.TileContext(nc) as tc:
        with (
            tc.tile_pool(name="wx", bufs=1) as wxpool,
            tc.tile_pool(name="consts", bufs=1) as cpool,
            tc.tile_pool(name="xc8", bufs=1) as xc8pool,
            tc.tile_pool(name="xcE8", bufs=1) as xcE8pool,
            tc.tile_pool(name="ps", bufs=8, space="PSUM") as pspool,
        ):
            # x_proj weights arrive during phase A; declared first so the
            # pool exists, DMA trigger is emitted after the phase-A loads.
            wx_t = wxpool.tile([128, KC, D_INNER], fp8, name="wx_t")
            xc8_t = xc8pool.tile([128, KC, T], fp8, name="xc8_t")
            xcE8_t = xcE8pool.tile([128, KC, T], fp8, name="xcE8_t")

            with (
                tc.tile_pool(name="xhl", bufs=1) as xhlpool,
                tc.tile_pool(name="wi", bufs=1) as wipool,
                tc.tile_pool(name="xi", bufs=2) as xipool,
                tc.tile_pool(name="taps", bufs=1) as tappool,
                tc.tile_pool(name="xcw", bufs=2) as xcwpool,
            ):
                xhl_t = xhlpool.tile([128, 2 * KI, T], fp8, name="xhl_t")
                wi_t = wipool.tile([128, KI, D_INNER], fp8, name="wi_t")
                # interleave weight-pair / x-pair loads so matmuls can start
                # as soon as the first pairs land
                for kp in range(KI // 2):
                    nc.sync.dma_start(
                        wi_t[:, 2 * kp:2 * kp + 2, :],
                        wi_d[:, 2 * kp * D_INNER:(2 * kp + 2) * D_INNER])
                    nc.sync.dma_start(
                        xhl_t[:, 2 * kp:2 * kp + 2, :],
                        xhl_d[:, 2 * kp * T:(2 * kp + 2) * T])
                    nc.sync.dma_start(
                        xhl_t[:, KI + 2 * kp:KI + 2 * kp + 2, :],
                        xhl_d[:, (KI + 2 * kp) * T:(KI + 2 * kp + 2) * T])
                wc_t = cpool.tile([128, CT * 4], f32, name="wc_t")
                nc.sync.dma_start(wc_t[:], wc_d[:, :])
                bc_t = cpool.tile([128, CT], f32, name="bc_t")
                nc.sync.dma_start(bc_t[:], bc_d[:, :])
                bx_t = cpool.tile([128, CT], f32, name="bx_t")
                nc.sync.dma_start(bx_t[:], bx_d[:, :])
                bo_t = cpool.tile([128, DT], f32, name="bo_t")
                nc.sync.dma_start(bo_t[:], bo_d[:, :])
                h0_t = cpool.tile([128, CT * 3], f32, name="h0_t")
                nc.sync.dma_start(h0_t[:], h0_d[:, :])
                nc.sync.dma_start(wx_t[:], wx_d[:, :])

                # ---- phase A: in_proj + conv + silu + quantize ----
                for ct in range(CT):
                    ps_j = [pspool.tile([128, W], f32, tag="ps",
                                        name=f"psa{ct}_{j}") for j in range(NJ)]
                    for kp in range(KI // 2):
                        wsl = wi_t[:, 2 * kp:2 * kp + 2,
                                   ct * 128:(ct + 1) * 128]
                        for part in range(2):
                            base = part * KI + 2 * kp
                            for j in range(NJ):
                                nc.tensor.matmul(
                                    ps_j[j][:], wsl,
                                    xhl_t[:, base:base + 2,
                                          j * W:(j + 1) * W],
                                    start=(kp == 0 and part == 0),
                                    stop=(kp == KI // 2 - 1 and part == 1),
                                    perf_mode=DR)
                    xi_t = xipool.tile([128, 3 + T], fp16, tag="xi",
                                       name=f"xi{ct}")
                    nc.vector.tensor_copy(
                        xi_t[:, 0:3], h0_t[:, ct * 3:ct * 3 + 3])
                    for j in range(NJ):
                        nc.vector.tensor_copy(
                            xi_t[:, 3 + j * W:3 + (j + 1) * W], ps_j[j][:])
                    p = []
                    for k in range(4):
                        pk = tappool.tile([128, T], fp16, tag=f"p{k}",
                                          bufs=(2 if k == 0 else 1),
                                          name=f"p{k}_{ct}")
                        nc.vector.tensor_scalar_mul(
                            pk[:], xi_t[:, k:k + T],
                            wc_t[:, ct * 4 + k:ct * 4 + k + 1])
                        p.append(pk)
                    nc.vector.tensor_tensor(p[0][:], p[0][:], p[1][:],
                                            op=ALU.add)
                    nc.vector.tensor_tensor(p[2][:], p[2][:], p[3][:],
                                            op=ALU.add)
                    nc.vector.tensor_tensor(p[0][:], p[0][:], p[2][:],
                                            op=ALU.add)
                    xc_t = xcwpool.tile([128, T], fp16, tag="xc",
                                        name=f"xc{ct}")
                    nc.scalar.activation(xc_t[:], p[0][:], AF.Silu,
                                         bias=bc_t[:, ct:ct + 1], scale=1.0)
                    nc.vector.tensor_scalar_mul(
                        xc8_t[:, ct, :], xc_t[:], SC)
                    xcE_t = xcwpool.tile([128, T], fp16, tag="xcE",
                                         name=f"xcE{ct}")
                    nc.vector.scalar_tensor_tensor(
                        xcE_t[:], xc_t[:], SC, xc8_t[:, ct, :],
                        op0=ALU.mult, op1=ALU.subtract)
                    nc.gpsimd.tensor_copy(xcE8_t[:, ct, :], xcE_t[:])

            # ---- phases B + C ----
            with (
                tc.tile_pool(name="y8", bufs=1) as y8pool,
                tc.tile_pool(name="wo", bufs=1) as wopool,
                tc.tile_pool(name="gate", bufs=2) as gatepool,
                tc.tile_pool(name="yw", bufs=2) as ywpool,
                tc.tile_pool(name="out", bufs=4) as opool,
            ):
                wo_t = wopool.tile([128, KC, D_MODEL], fp8, name="wo_t")
                nc.sync.dma_start(wo_t[:], wo_d[:, :])
                yhi_t = y8pool.tile([128, KC, T], fp8, name="yhi_t")
                ylo_t = y8pool.tile([128, KC, T], fp8, name="ylo_t")

                # ---- phase B: x_proj + gate + y + quantize ----
                for c2 in range(CT):
                    ps_j = [pspool.tile([128, W], f32, tag="ps",
                                        name=f"psb{c2}_{j}") for j in range(NJ)]
                    for kp in range(KC // 2):
                        wsl = wx_t[:, 2 * kp:2 * kp + 2,
                                   c2 * 128:(c2 + 1) * 128]
                        for j in range(NJ):
                            nc.tensor.matmul(
                                ps_j[j][:], wsl,
                                xc8_t[:, 2 * kp:2 * kp + 2,
                                      j * W:(j + 1) * W],
                                start=(kp == 0), stop=(kp == KC // 2 - 1),
                                perf_mode=DR)
                    gate_t = gatepool.tile([128, T], fp16, tag="g",
                                           name=f"g{c2}")
                    for j in range(NJ):
                        nc.scalar.activation(
                            gate_t[:, j * W:(j + 1) * W], ps_j[j][:],
                            AF.Sigmoid, bias=bx_t[:, c2:c2 + 1],
                            scale=s_x / SC)
                    y_t = ywpool.tile([128, T], fp16, tag="y", name=f"y{c2}")
                    nc.vector.tensor_tensor(
                        y_t[:], xc8_t[:, c2, :], xcE8_t[:, c2, :], op=ALU.add)
                    nc.vector.tensor_tensor(y_t[:], y_t[:], gate_t[:],
                                            op=ALU.mult)
                    nc.vector.tensor_copy(yhi_t[:, c2, :], y_t[:])
                    nc.vector.tensor_tensor(
                        y_t[:], y_t[:], yhi_t[:, c2, :], op=ALU.subtract)
                    nc.gpsimd.tensor_copy(ylo_t[:, c2, :], y_t[:])

                # ---- phase C: out_proj ----
                for dt in range(DT):
                    ps_j = [pspool.tile([128, W], f32, tag="ps",
                                        name=f"psc{dt}_{j}") for j in range(NJ)]
                    for kp in range(KC // 2):
                        wsl = wo_t[:, 2 * kp:2 * kp + 2,
                                   dt * 128:(dt + 1) * 128]
                        for part, src in enumerate((yhi_t, ylo_t)):
                            for j in range(NJ):
                                nc.tensor.matmul(
                                    ps_j[j][:], wsl,
                                    src[:, 2 * kp:2 * kp + 2,
                                        j * W:(j + 1) * W],
                                    start=(kp == 0 and part == 0),
                                    stop=(kp == KC // 2 - 1 and part == 1),
                                    perf_mode=DR)
                    for j in range(NJ):
                        ot = opool.tile([128, W], bf16, tag="ot",
                                        name=f"ot{dt}_{j}")
                        nc.scalar.activation(ot[:], ps_j[j][:], AF.Identity,
                                             bias=bo_t[:, dt:dt + 1],
                                             scale=s_out / SC)
                        nc.sync.dma_start(
                            out_d[:, dt * T + j * W:dt * T + (j + 1) * W],
                            ot[:])

    nc.compile()
    return nc


def _quantize(w):
    s = np.float32(max(np.abs(w).mean(dtype=np.float64), EPS))
    return np.clip(np.round(w / s), -1.0, 1.0).astype(np.float32), s


def _plane_pack(a, nplanes, width):
    """[nplanes*128, width] -> [128, nplanes*width] with plane-major cols."""
    return np.ascontiguousarray(
        a.reshape(nplanes, 128, width).transpose(1, 0, 2).reshape(
            128, nplanes * width))


def kernel(x, w_in, b_in, w_conv, b_conv, w_x, b_x, w_out, b_out,
           _trace=False, _trace_kwargs=None):
    from concourse import bass_utils

    x = np.asarray(x, dtype=np.float32)
    w_in = np.asarray(w_in, dtype=np.float32)
    b_in = np.asarray(b_in, dtype=np.float32)
    w_conv = np.asarray(w_conv, dtype=np.float32)
    b_conv = np.asarray(b_conv, dtype=np.float32)
    w_x = np.asarray(w_x, dtype=np.float32)
    b_x = np.asarray(b_x, dtype=np.float32)
    w_out = np.asarray(w_out, dtype=np.float32)
    b_out = np.asarray(b_out, dtype=np.float32)

    # ---- host-side BitNet quantization (exact ternary) ----
    wq_in, s_in = _quantize(w_in)     # [2*D_INNER, D_MODEL]
    wq_x, s_x = _quantize(w_x)        # [D_STATE+D_MODEL+D_INNER, D_INNER]
    wq_out, s_out = _quantize(w_out)  # [D_MODEL, D_INNER]
    wq_in = wq_in[:D_INNER]           # res half unused downstream
    wq_x_d = wq_x[:D_INNER]           # only delta rows used

    fp8 = ml_dtypes.float8_e4m3
    wi_pk = _plane_pack(np.ascontiguousarray(wq_in.T), KI, D_INNER).astype(fp8)
    wx_pk = _plane_pack(np.ascontiguousarray(wq_x_d.T), KC,
                        D_INNER).astype(fp8)
    wo_pk = _plane_pack(np.ascontiguousarray(wq_out.T), KC,
                        D_MODEL).astype(fp8)

    # conv taps with in_proj scale folded in; bias absorbs b_in
    wc = (s_in * w_conv[:, 0, :]).astype(np.float32)             # [D_INNER, 4]
    bc = (b_in[:D_INNER] * w_conv[:, 0, :].sum(axis=1)
          + b_conv).astype(np.float32)
    wc_pk = np.ascontiguousarray(
        wc.reshape(CT, 128, 4).transpose(1, 0, 2).reshape(128, CT * 4))
    bc_pk = _plane_pack(bc, CT, 1)
    bx_pk = _plane_pack(b_x[:D_INNER].astype(np.float32), CT, 1)
    bo_pk = _plane_pack(b_out.astype(np.float32), DT, 1)

    # ---- shard inputs: hi+lo fp8 of x^T ----
    x_flat = x.reshape(B * S, D_MODEL)
    xT = np.ascontiguousarray(x_flat.T)                   # [D_MODEL, B*S] f32
    x_hi = xT.astype(fp8)
    x_lo = (xT - x_hi.astype(np.float32)).astype(fp8)

    # raw in_proj value that makes x_inner == 0 (sequence-start padding)
    pad_raw = (-b_in[:D_INNER] / s_in).astype(np.float32)

    in_maps = []
    for c in range(N_CORES):
        t0 = c * T
        hi = _plane_pack(x_hi[:, t0:t0 + T], KI, T)
        lo = _plane_pack(x_lo[:, t0:t0 + T], KI, T)
        xhl = np.ascontiguousarray(np.concatenate([hi, lo], axis=1))
        if t0 % S == 0:
            h0 = np.repeat(pad_raw[:, None], 3, axis=1)   # [D_INNER, 3]
        else:
            h0 = wq_in @ x_flat[t0 - 3:t0].T              # [D_INNER, 3]
        h0_pk = _plane_pack(h0.astype(np.float32), CT, 3)
        in_maps.append({
            "xhl": xhl, "wi": wi_pk, "wx": wx_pk, "wo": wo_pk,
            "wc": wc_pk, "bc": bc_pk, "bx": bx_pk, "bo": bo_pk,
            "h0": h0_pk,
        })

    key = (float(s_x), float(s_out))
    if key not in _BUILD_CACHE:
        _BUILD_CACHE[key] = _build(float(s_x), float(s_out))
    nc = _BUILD_CACHE[key]

    kwargs = {}
    if _trace:
        kwargs["trace"] = True
        if _trace_kwargs:
            kwargs.update(_trace_kwargs)
    res = bass_utils.run_bass_kernel_spmd(
        nc, in_maps, core_ids=list(range(N_CORES)), **kwargs)
    kernel.last_results = res

    outs = []
    for c in range(N_CORES):
        arr = np.asarray(res.results[c]["out"]).astype(np.float32)
        outs.append(arr.reshape(128, DT, T).transpose(1, 0, 2).reshape(
            D_MODEL, T))
    full = np.concatenate(outs, axis=1)                   # [D_MODEL, B*S]
    return np.ascontiguousarray(full.T).reshape(B, S, D_MODEL).astype(
        np.float32)


# revision 6
# speedup vs baseline: 1.8084x; 1.8084x over previous
"""BitSSM fused kernel for 8 Trainium2 NeuronCores.

Strategy
--------
Data-parallel over tokens: B*S = 16384 tokens split into 8 shards of 2048.
All ops are token-local except the causal depthwise conv (K=4), whose
3-token left halo is precomputed on the host per shard.

PE does all heavy math (one 512-token moving pass costs the same 216ns
regardless of dtype; fp8 DoubleRow contracts 2 K-planes per pass):
  in_proj : fp16 moving x, fp8 ternary stationary      (8 planes / ct-tile)
  conv    : 4 shifted fp16 matmuls w/ diagonal stationary, accumulated
            into one PSUM group (K=4 causal depthwise conv)
  x_proj  : fp8 DoubleRow over X8 = fp8(64*xc)         (16 planes -> 8 MMs)
  out_proj: fp8 DoubleRow over X8 and B'               (32 planes -> 16 MMs)
            where B' = fp8(128*xc*gate - X8), so X8 + B' = 128*y with only
            a small-residual fp8 quantization error.

Phases per core (token halves H=1024 keep SBUF under budget):
  A(h): in_proj + conv + silu -> xc (fp16, stored) -> X8 (fp8, stored)
  B(h): x_proj -> gate = sigmoid(s_x/64 * psum + bx) -> t = xc*g ->
        B' = fp8(128*t - X8)
  C   : out_proj over (X8, B') pairs; out = Identity(s_out/128*psum + bo)
"""

import sys

if '/opt/trn_rl_repo' not in sys.path:
    sys.path.insert(0, '/opt/trn_rl_repo')

import numpy as np
import ml_dtypes

D_MODEL, D_STATE, D_INNER = 1024, 16, 2048
EPS = 1e-5
B, S = 4, 4096
N_CORES = 8
T = (B * S) // N_CORES          # tokens per core (2048)
H = T // 2                      # tokens per phase half (1024)
W = 512                         # psum tile width (tokens)
KI = D_MODEL // 128             # 8 contraction planes for in_proj
KC = D_INNER // 128             # 16 contraction planes for x/out_proj
CT = D_INNER // 128             # 16 channel planes of d_inner
DT = D_MODEL // 128             # 8 channel planes of d_model
SC = 64.0                       # fp8 scale for xc

_BUILD_CACHE = {}


def _build(s_x: float, s_out: float):
    import concourse.tile as tile
    from concourse import bacc, mybir

    nc = bacc.Bacc("TRN2", target_bir_lowering=False, debug=False)
    f32 = mybir.dt.float32
    fp16 = mybir.dt.float16
    bf16 = mybir.dt.bfloat16
    fp8 = mybir.dt.float8e4
    AF = mybir.ActivationFunctionType
    ALU = mybir.AluOpType
    DR = mybir.MatmulPerfMode.DoubleRow

    x16_d = nc.dram_tensor("x16", [128, KI, T], fp16, kind="ExternalInput")
    wi_d = nc.dram_tensor("wi", [128, KI * D_INNER], fp8, kind="ExternalInput")
    wx_d = nc.dram_tensor("wx", [128, KC * D_INNER], fp8, kind="ExternalInput")
    wo_d = nc.dram_tensor("wo", [128, KC * D_MODEL], fp8, kind="ExternalInput")
    dg_d = nc.dram_tensor("dg", [128, CT * 4 * 128], fp16, kind="ExternalInput")
    bc_d = nc.dram_tensor("bc", [128, CT], f32, kind="ExternalInput")
    bx_d = nc.dram_tensor("bx", [128, CT], f32, kind="ExternalInput")
    bo_d = nc.dram_tensor("bo", [128, DT], f32, kind="ExternalInput")
    h0_d = nc.dram_tensor("h0", [128, CT * 3], f32, kind="ExternalInput")
    out_d = nc.dram_tensor("out", [128, DT * T], bf16, kind="ExternalOutput")

    with tile.TileContext(nc) as tc:
        with (
            tc.tile_pool(name="wx", bufs=1) as wxpool,
            tc.tile_pool(name="x8", bufs=1) as x8pool,
            tc.tile_pool(name="consts", bufs=1) as cpool,
            tc.tile_pool(name="ps", bufs=8, space="PSUM") as pspool,
        ):
            wx_t = wxpool.tile([128, KC, D_INNER], fp8, name="wx_t")
            X8_t = x8pool.tile([128, KC, T], fp8, name="X8_t")
            BP_t = x8pool.tile([128, KC, T], fp8, name="BP_t")

            with (
                tc.tile_pool(name="xin", bufs=1) as xinpool,
                tc.tile_pool(name="wi", bufs=1) as wipool,
                tc.tile_pool(name="dg", bufs=1) as dgpool,
                tc.tile_pool(name="xi", bufs=2) as xipool,
                tc.tile_pool(name="xc", bufs=1) as xcpool,
                tc.tile_pool(name="gate", bufs=2) as gatepool,
                tc.tile_pool(name="tw", bufs=2) as twpool,
            ):
                wi_t = wipool.tile([128, KI, D_INNER], fp8, name="wi_t")
                nc.sync.dma_start(wi_t[:], wi_d[:, :])
                dg_t = dgpool.tile([128, CT * 4, 128], fp16, name="dg_t")
                nc.sync.dma_start(dg_t[:], dg_d[:, :])
                bc_t = cpool.tile([128, CT], f32, name="bc_t")
                nc.sync.dma_start(bc_t[:], bc_d[:, :])
                bx_t = cpool.tile([128, CT], f32, name="bx_t")
                nc.sync.dma_start(bx_t[:], bx_d[:, :])
                bo_t = cpool.tile([128, DT], f32, name="bo_t")
                nc.sync.dma_start(bo_t[:], bo_d[:, :])
                h0_t = cpool.tile([128, CT * 3], f32, name="h0_t")
                nc.sync.dma_start(h0_t[:], h0_d[:, :])
                halo1 = cpool.tile([128, CT * 3], fp16, name="halo1")

                def phase_A(h):
                    x16_t = xinpool.tile([128, KI, H], fp16, tag="x16",
                                         name=f"x16_{h}")
                    nc.sync.dma_start(
                        x16_t[:], x16_d[:, :, h * H:(h + 1) * H])
                    if h == 0:
                        nc.sync.dma_start(wx_t[:], wx_d[:, :])
                    phase_A.xc_t = xcpool.tile([128, CT, H], fp16, tag="xc",
                                               name=f"xc_{h}")
                    pend = []          # (ct, in_ps pair) awaiting conv

                    def conv_group(ct, ps_in):
                        xi_t = xipool.tile([128, 3 + H], fp16, tag="xi",
                                           name=f"xi{ct}_{h}")
                        if h == 0:
                            nc.vector.tensor_copy(
                                xi_t[:, 0:3], h0_t[:, ct * 3:ct * 3 + 3])
                        else:
                            nc.vector.tensor_copy(
                                xi_t[:, 0:3], halo1[:, ct * 3:ct * 3 + 3])
                        for jh in range(2):
                            nc.vector.tensor_copy(
                                xi_t[:, 3 + jh * W:3 + (jh + 1) * W],
                                ps_in[jh][:])
                        if h == 0:
                            nc.vector.tensor_copy(
                                halo1[:, ct * 3:ct * 3 + 3],
                                xi_t[:, H:H + 3])
                        ps_cv = [pspool.tile([128, W], f32, tag="ps",
                                             name=f"pscv{ct}_{jh}_{h}")
                                 for jh in range(2)]
                        for k in range(4):
                            dsl = dg_t[:, ct * 4 + k, :]
                            for jh in range(2):
                                nc.tensor.matmul(
                                    ps_cv[jh][:], dsl,
                                    xi_t[:, k + jh * W:k + jh * W + W],
                                    start=(k == 0), stop=(k == 3))
                        for jh in range(2):
                            nc.scalar.activation(
                                phase_A.xc_t[:, ct, jh * W:(jh + 1) * W],
                                ps_cv[jh][:], AF.Silu,
                                bias=bc_t[:, ct:ct + 1], scale=1.0)
                        nc.vector.tensor_scalar_mul(
                            X8_t[:, ct, h * H:(h + 1) * H],
                            phase_A.xc_t[:, ct, :], SC)

                    for ct in range(CT):
                        ps_in = [pspool.tile([128, W], f32, tag="ps",
                                             name=f"psin{ct}_{jh}_{h}")
                                 for jh in range(2)]
                        for kt in range(KI):
                            wsl = wi_t[:, kt, ct * 128:(ct + 1) * 128]
                            for jh in range(2):
                                nc.tensor.matmul(
                                    ps_in[jh][:], wsl,
                                    x16_t[:, kt, jh * W:(jh + 1) * W],
                                    start=(kt == 0), stop=(kt == KI - 1))
                        pend.append((ct, ps_in))
                        if len(pend) > 1:
                            conv_group(*pend.pop(0))
                    conv_group(*pend.pop(0))

                def phase_B(h):
                    xc_t = phase_A.xc_t
                    for c2 in range(CT):
                        ps_j = [pspool.tile([128, W], f32, tag="ps",
                                            name=f"psb{c2}_{jh}_{h}")
                                for jh in range(2)]
                        for kp in range(KC // 2):
                            wsl = wx_t[:, 2 * kp:2 * kp + 2,
                                       c2 * 128:(c2 + 1) * 128]
                            for jh in range(2):
                                nc.tensor.matmul(
                                    ps_j[jh][:], wsl,
                                    X8_t[:, 2 * kp:2 * kp + 2,
                                         h * H + jh * W:h * H + (jh + 1) * W],
                                    start=(kp == 0), stop=(kp == KC // 2 - 1),
                                    perf_mode=DR)
                        gate_t = gatepool.tile([128, H], fp16, tag="g",
                                               name=f"g{c2}_{h}")
                        for jh in range(2):
                            nc.scalar.activation(
                                gate_t[:, jh * W:(jh + 1) * W], ps_j[jh][:],
                                AF.Sigmoid, bias=bx_t[:, c2:c2 + 1],
                                scale=s_x / SC)
                        t_t = twpool.tile([128, H], fp16, tag="t",
                                          name=f"t{c2}_{h}")
                        nc.vector.tensor_tensor(
                            t_t[:], xc_t[:, c2, :], gate_t[:], op=ALU.mult)
                        nc.vector.scalar_tensor_tensor(
                            BP_t[:, c2, h * H:(h + 1) * H], t_t[:], 2 * SC,
                            X8_t[:, c2, h * H:(h + 1) * H],
                            op0=ALU.mult, op1=ALU.subtract)

                phase_A(0)
                phase_B(0)
                phase_A(1)
                phase_B(1)

            # ---- phase C: out_proj ----
            with (
                tc.tile_pool(name="wo", bufs=1) as wopool,
                tc.tile_pool(name="out", bufs=4) as opool,
            ):
                wo_t = wopool.tile([128, KC, D_MODEL], fp8, name="wo_t")
                nc.sync.dma_start(wo_t[:], wo_d[:, :])
                for dt in range(DT):
                    ps_j = [pspool.tile([128, W], f32, tag="ps",
                                        name=f"psc{dt}_{j}") for j in range(4)]
                    for src_i, src in enumerate((X8_t, BP_t)):
                        for kp in range(KC // 2):
                            wsl = wo_t[:, 2 * kp:2 * kp + 2,
                                       dt * 128:(dt + 1) * 128]
                            for j in range(4):
                                nc.tensor.matmul(
                                    ps_j[j][:], wsl,
                                    src[:, 2 * kp:2 * kp + 2,
                                        j * W:(j + 1) * W],
                                    start=(src_i == 0 and kp == 0),
                                    stop=(src_i == 1 and kp == KC // 2 - 1),
                                    perf_mode=DR)
                    for j in range(4):
                        ot = opool.tile([128, W], bf16, tag="ot",
                                        name=f"ot{dt}_{j}")
                        nc.scalar.activation(ot[:], ps_j[j][:], AF.Identity,
                                             bias=bo_t[:, dt:dt + 1],
                                             scale=s_out / (2 * SC))
                        nc.sync.dma_start(
                            out_d[:, dt * T + j * W:dt * T + (j + 1) * W],
                            ot[:])

    nc.compile()
    return nc


def _quantize(w):
    s = np.float32(max(np.abs(w).mean(dtype=np.float64), EPS))
    return np.clip(np.round(w / s), -1.0, 1.0).astype(np.float32), s


def _plane_pack(a, nplanes, width):
    """[nplanes*128, width] -> [128, nplanes*width] with plane-major cols."""
    return np.ascontiguousarray(
        a.reshape(nplanes, 128, width).transpose(1, 0, 2).reshape(
            128, nplanes * width))


def kernel(x, w_in, b_in, w_conv, b_conv, w_x, b_x, w_out, b_out,
           _trace=False, _trace_kwargs=None):
    from concourse import bass_utils

    x = np.asarray(x, dtype=np.float32)
    w_in = np.asarray(w_in, dtype=np.float32)
    b_in = np.asarray(b_in, dtype=np.float32)
    w_conv = np.asarray(w_conv, dtype=np.float32)
    b_conv = np.asarray(b_conv, dtype=np.float32)
    w_x = np.asarray(w_x, dtype=np.float32)
    b_x = np.asarray(b_x, dtype=np.float32)
    w_out = np.asarray(w_out, dtype=np.float32)
    b_out = np.asarray(b_out, dtype=np.float32)

    # ---- host-side BitNet quantization (exact ternary) ----
    wq_in, s_in = _quantize(w_in)
    wq_x, s_x = _quantize(w_x)
    wq_out, s_out = _quantize(w_out)
    wq_in = wq_in[:D_INNER]           # res half unused downstream
    wq_x_d = wq_x[:D_INNER]           # only delta rows used

    fp8 = ml_dtypes.float8_e4m3
    fp16 = np.float16
    wi_pk = _plane_pack(np.ascontiguousarray(wq_in.T), KI, D_INNER).astype(fp8)
    wx_pk = _plane_pack(np.ascontiguousarray(wq_x_d.T), KC,
                        D_INNER).astype(fp8)
    wo_pk = _plane_pack(np.ascontiguousarray(wq_out.T), KC,
                        D_MODEL).astype(fp8)

    # conv taps as 64 diagonal [128,128] fp16 stationary blocks (ct-major, k)
    wc = (s_in * w_conv[:, 0, :]).astype(np.float32)             # [D_INNER, 4]
    dg = np.zeros((128, CT * 4, 128), dtype=fp16)
    wc16 = wc.astype(fp16)
    for ct in range(CT):
        for k in range(4):
            np.fill_diagonal(dg[:, ct * 4 + k, :], wc16[ct * 128:(ct + 1) * 128, k])
    dg_pk = np.ascontiguousarray(dg.reshape(128, CT * 4 * 128))

    bc = (b_in[:D_INNER] * w_conv[:, 0, :].sum(axis=1)
          + b_conv).astype(np.float32)
    bc_pk = _plane_pack(bc, CT, 1)
    bx_pk = _plane_pack(b_x[:D_INNER].astype(np.float32), CT, 1)
    bo_pk = _plane_pack(b_out.astype(np.float32), DT, 1)

    # ---- shard inputs: x^T in fp16 ----
    x_flat = x.reshape(B * S, D_MODEL)
    xT = np.ascontiguousarray(x_flat.T)                   # [D_MODEL, B*S] f32
    xT16 = xT.astype(fp16)

    # raw in_proj value that makes x_inner == 0 (sequence-start padding)
    pad_raw = (-b_in[:D_INNER] / s_in).astype(np.float32)

    in_maps = []
    for c in range(N_CORES):
        t0 = c * T
        x16 = _plane_pack(xT16[:, t0:t0 + T], KI, T).reshape(128, KI, T)
        if t0 % S == 0:
            h0 = np.repeat(pad_raw[:, None], 3, axis=1)   # [D_INNER, 3]
        else:
            h0 = wq_in @ x_flat[t0 - 3:t0].T              # [D_INNER, 3]
        h0_pk = _plane_pack(h0.astype(np.float32), CT, 3)
        in_maps.append({
            "x16": x16, "wi": wi_pk, "wx": wx_pk, "wo": wo_pk,
            "dg": dg_pk, "bc": bc_pk, "bx": bx_pk, "bo": bo_pk,
            "h0": h0_pk,
        })

    key = (float(s_x), float(s_out))
    if key not in _BUILD_CACHE:
        _BUILD_CACHE[key] = _build(float(s_x), float(s_out))
    nc = _BUILD_CACHE[key]

    kwargs = {}
    if _trace:
        kwargs["trace"] = True
        if _trace_kwargs:
            kwargs.update(_trace_kwargs)
    res = bass_utils.run_bass_kernel_spmd(
        nc, in_maps, core_ids=list(range(N_CORES)), **kwargs)
    kernel.last_results = res

    outs = []
    for c in range(N_CORES):
        arr = np.asarray(res.results[c]["out"]).astype(np.float32)
        outs.append(arr.reshape(128, DT, T).transpose(1, 0, 2).reshape(
            D_MODEL, T))
    full = np.concatenate(outs, axis=1)                   # [D_MODEL, B*S]
    return np.ascontiguousarray(full.T).reshape(B, S, D_MODEL).astype(
        np.float32)
